# revision 15
# baseline (speedup 1.0000x reference)
# CARAFE Trainium2 Bass kernel, band-matrix formulation.
# Data-parallel over batch (8 items -> 8 NeuronCores). Per core:
#   - 1x1 compressor conv + folded BN + SiLU (PE fp16 matmuls, ACT sigmoid,
#     DVE fused mul)
#   - 3x3 encoder conv as 9 accumulating fp16 matmuls + folded BN, evicted
#     through ACT Exp -> unnormalized masks (bf16)
#   - mask normalization: per-pixel sum via a ones-selector matmul (PE),
#     reciprocal on DVE, replicated across the 100 mask channels by DMA
#   - per-tap-offset shifted mask copies (25 sbuf->sbuf DMAs into a padded
#     tile), transposed into source-pixel-major W tiles (PE)
#   - gpsimd local_scatter builds, per 128-pixel output block, the sparse
#     band matrix A[source_pixel, output_position] holding the 25 mask
#     diagonals; the whole 25-tap reassembly then collapses into 3
#     accumulating PE matmuls per (block, channel-chunk): out = x_pixmajor^T @ A
#   - PSUM evicted (ACT/DVE) to bf16 staging, DMA'd to DRAM channel-major.
import sys
import numpy as np

for _p in ("/opt/trn_rl_repo",):
    if _p not in sys.path:
        sys.path.insert(0, _p)

import ml_dtypes

B, C, Cm, E = 8, 192, 64, 100
H = W = 64
PIX = H * W              # 4096
K, S = 5, 2
EPS = 1e-3
NMM = 34                 # padded 128-pixel row tiles (mm = m + 1, m in [-1,33))
NTB = 32                 # output blocks of 2 lowres rows
EPAD = 384               # zero pad each side of the normalized-mask tile
ENW = EPAD + PIX + EPAD  # 4864
ESW = NMM * 128          # 4352
NBLK = 1                 # blocks staged per output DMA

_prog_cache = {}


def _off(i, j):
    return 64 * (i - 2) + (j - 2)


def _make_idx():
    idx = np.full((128, 300), -1, np.int16)
    for sp in range(128):
        for kp in range(3):
            for i in range(5):
                for j in range(5):
                    off = _off(i, j)
                    p_rel = 128 * (kp - 1) + sp - off
                    if not (0 <= p_rel < 128):
                        continue
                    r, w = divmod(p_rel, 64)
                    for cl in range(4):
                        di, dj = divmod(cl, 2)
                        col = kp * 100 + (i * 5 + j) * 4 + cl
                        idx[sp, col] = kp * 512 + ((r * 2 + di) * 64 + w) * 2 + dj
    return idx


def _build_program(num_devices=8):
    import concourse.mybir as mybir
    import concourse.tile as tile
    from concourse import bacc
    from contextlib import ExitStack

    fp32 = mybir.dt.float32
    fp16 = mybir.dt.float16
    bf16 = mybir.dt.bfloat16
    i16 = mybir.dt.int16

    nc = bacc.Bacc("TRN2", target_bir_lowering=False, num_devices=num_devices)

    x0_d = nc.dram_tensor("x0", [128, PIX], fp16, kind="ExternalInput").ap()
    x1_d = nc.dram_tensor("x1", [64, PIX], fp16, kind="ExternalInput").ap()
    cw0_d = nc.dram_tensor("cw0", [128, Cm], fp16, kind="ExternalInput").ap()
    cw1_d = nc.dram_tensor("cw1", [64, Cm], fp16, kind="ExternalInput").ap()
    cb_d = nc.dram_tensor("cb", [Cm, 1], fp32, kind="ExternalInput").ap()
    ew_d = nc.dram_tensor("ew", [Cm, 9 * E], fp16, kind="ExternalInput").ap()
    eb_d = nc.dram_tensor("eb", [E, 1], fp32, kind="ExternalInput").ap()
    out_d = nc.dram_tensor("out", [C, 2 * H, 2 * W], bf16, kind="ExternalOutput").ap()
    e_dram = nc.dram_tensor("edrm", [E, ENW], fp16, kind="Internal").ap()

    identh_t = nc.inline_tensor(np.eye(128, dtype=np.float16), name="identh").ap()
    sel_np = np.zeros((E, E), np.float32)
    for ci in range(E):
        for co in range(E):
            if ci % 4 == co % 4:
                sel_np[ci, co] = 1.0
    sel_t = nc.inline_tensor(sel_np.astype(ml_dtypes.bfloat16), name="selc").ap()
    edge_np = np.zeros((128, E), np.float32)
    for sp in range(128):
        for ch in range(E):
            j = (ch // 4) % 5
            wp_ = (sp - j + 2) % 64
            edge_np[sp, ch] = 1.0 if 0 <= wp_ + j - 2 < W else 0.0
    edge_t = nc.inline_tensor(edge_np.astype(np.float16), name="edgec").ap()
    idx_t = nc.inline_tensor(_make_idx(), name="idxc").ap()

    es = ExitStack()
    with tile.TileContext(nc) as tc:
        with es:
            _body(es, tc, nc, mybir, fp32, fp16, bf16, i16,
                  x0_d, x1_d, cw0_d, cw1_d, cb_d, ew_d, eb_d, out_d, e_dram,
                  identh_t, sel_t, edge_t, idx_t)
    nc.compile()
    return nc


def _body(es, tc, nc, mybir, fp32, fp16, bf16, i16,
          x0_d, x1_d, cw0_d, cw1_d, cb_d, ew_d, eb_d, out_d, e_dram,
          identh_t, sel_t, edge_t, idx_t):
    from contextlib import ExitStack
    AL = mybir.AluOpType
    AF = mybir.ActivationFunctionType

    consts = es.enter_context(tc.tile_pool(name="consts", bufs=1))
    big = es.enter_context(tc.tile_pool(name="big", bufs=1))

    identh = consts.tile([128, 128], fp16, tag="identh")
    sel = consts.tile([E, E], bf16, tag="sel")
    edge = consts.tile([128, E], fp16, tag="edge")
    idxt = consts.tile([128, 300], i16, tag="idxt")
    cw0 = consts.tile([128, Cm], fp16, tag="cw0")
    cw1 = consts.tile([64, Cm], fp16, tag="cw1")
    cb = consts.tile([Cm, 1], fp32, tag="cb")
    ew = consts.tile([Cm, 9 * E], fp16, tag="ew")
    eb = consts.tile([E, 1], fp32, tag="eb")

    x0 = big.tile([128, PIX], fp16, tag="x0")
    x1 = big.tile([64, PIX], fp16, tag="x1")
    xT = big.tile([128, NMM, C], fp16, tag="xT")
    tpad = big.tile([Cm, 66 * 66], fp16, tag="tpad")
    eraw = big.tile([E, PIX], bf16, tag="eraw")
    zrep = big.tile([E, PIX], bf16, tag="zrep")
    enp = big.tile([E, ENW], fp16, tag="enp")
    esh = big.tile([E, ESW], fp16, tag="esh")
    wn = big.tile([128, NMM, E], fp16, tag="wn")

    nc.scalar.dma_start(out=identh[:], in_=identh_t)
    nc.sync.dma_start(out=cw0[:], in_=cw0_d)
    nc.sync.dma_start(out=cw1[:], in_=cw1_d)
    nc.sync.dma_start(out=cb[:], in_=cb_d)
    nc.sync.dma_start(out=x0[:, 0:2048], in_=x0_d[:, 0:2048])
    nc.scalar.dma_start(out=x1[:, 0:2048], in_=x1_d[:, 0:2048])
    nc.sync.dma_start(out=x0[:, 2048:PIX], in_=x0_d[:, 2048:PIX])
    nc.scalar.dma_start(out=x1[:, 2048:PIX], in_=x1_d[:, 2048:PIX])
    nc.scalar.dma_start(out=ew[:], in_=ew_d)
    nc.scalar.dma_start(out=eb[:], in_=eb_d)
    nc.scalar.dma_start(out=sel[:], in_=sel_t)
    nc.scalar.dma_start(out=edge[:], in_=edge_t)
    nc.scalar.dma_start(out=idxt[:], in_=idx_t)

    nc.vector.memset(xT[:, 0, :], 0.0)
    nc.vector.memset(xT[:, NMM - 1, :], 0.0)
    tp3 = tpad[:].rearrange("c (r z) -> c r z", z=66)
    nc.vector.memset(tp3[:, 0:1, :], 0.0)
    nc.vector.memset(tp3[:, 65:66, :], 0.0)
    nc.vector.memset(tp3[:, 1:65, 0:1], 0.0)
    nc.vector.memset(tp3[:, 1:65, 65:66], 0.0)
    nc.vector.memset(enp[:, 0:EPAD], 0.0)
    nc.vector.memset(enp[:, EPAD + PIX:ENW], 0.0)

    c1ps = es.enter_context(tc.tile_pool(name="c1ps", bufs=2, space="PSUM"))
    c2ps = es.enter_context(tc.tile_pool(name="c2ps", bufs=2, space="PSUM"))
    trps = es.enter_context(tc.tile_pool(name="trps", bufs=2, space="PSUM"))
    outps = es.enter_context(tc.tile_pool(name="outps", bufs=1, space="PSUM"))
    apool = es.enter_context(tc.tile_pool(name="apool", bufs=8))
    stgp = es.enter_context(tc.tile_pool(name="stgp", bufs=6))

    def conv1(nt):
        n0 = nt * 512
        psf = c1ps.tile([128, 512], fp32, tag="c1")
        ps = psf[0:Cm, :]
        nc.tensor.matmul(ps, cw0[:], x0[:, n0:n0 + 512], start=True, stop=False)
        nc.tensor.matmul(ps, cw1[:], x1[:, n0:n0 + 512], start=False, stop=True)
        v = tp3[:, nt * 8 + 1:nt * 8 + 9, 1:65]
        nc.scalar.activation(out=v, in_=ps.rearrange("c (r z) -> c r z", z=64),
                             func=AF.Silu, bias=cb[:], scale=1.0)

    def conv2(nt):
        r0 = nt * 8
        ps = c2ps.tile([E, 512], fp32, tag="c2")
        for tap in range(9):
            dy, dx = divmod(tap, 3)
            rhs = tp3[:, r0 + dy:r0 + dy + 8, dx:dx + 64]
            nc.tensor.matmul(ps[:], ew[:, tap * E:(tap + 1) * E], rhs,
                             start=(tap == 0), stop=(tap == 8))
        nc.scalar.activation(out=eraw[:, nt * 512:(nt + 1) * 512], in_=ps[:],
                             func=AF.Exp, bias=eb[:], scale=1.0)

    def zblk(nt):
        pszf = c2ps.tile([E, 512], fp32, tag="c2")
        psz = pszf[:]
        nc.tensor.matmul(psz, sel[:], eraw[:, nt * 512:(nt + 1) * 512],
                         start=True, stop=True)
        with nc.allow_low_precision(reason="bf16 mask normalizer, 2e-2 tol"):
            nc.vector.reciprocal(zrep[:, nt * 512:(nt + 1) * 512], psz)

    def xtr(m, on_act=False):
        w0 = m * 128
        t0 = trps.tile([128, 128], fp16, tag="tp")
        nc.tensor.transpose(t0[:], x0[:, w0:w0 + 128], identh[:])
        t1f = trps.tile([128, 128], fp16, tag="tp")
        t1 = t1f[:, 0:64]
        nc.tensor.transpose(t1, x1[:, w0:w0 + 128], identh[0:64, 0:64])
        if on_act:
            nc.scalar.copy(out=xT[:, m + 1, 0:128], in_=t0[:])
            nc.scalar.copy(out=xT[:, m + 1, 128:192], in_=t1)
        else:
            nc.vector.tensor_scalar(xT[:, m + 1, 0:128], t0[:], 1.0, None, AL.mult)
            nc.vector.tensor_scalar(xT[:, m + 1, 128:192], t1, 1.0, None, AL.mult)

    for nt in range(8):
        conv1(nt)
        for sub in range(2):
            xtr(nt * 2 + sub)
    from concourse.ap import AP as _AP

    def wtile(mm):
        wpf = trps.tile([128, 128], fp16, tag="tp")
        wp = wpf[:, 0:E]
        nc.tensor.transpose(wp, esh[:, mm * 128:(mm + 1) * 128],
                            identh[0:E, 0:E])
        nc.vector.tensor_tensor(wn[:, mm, :], wp, edge[:], AL.mult)

    # staged shift roundtrip: (e_dram col range, esh q range, W mm range)
    BATCHES = {1: (0, 1408, 0, 896, 0, 7),
               4: (1408, 2944, 896, 2176, 7, 17),
               7: (2944, ENW, 2176, ESW, 17, NMM)}

    def shift_batch(nt):
        d0, d1, q0, q1, m0, m1 = BATCHES[nt]
        nc.sync.dma_start(out=e_dram[:, d0:d1], in_=enp[:, d0:d1])
        for i in range(5):
            base = (20 * i) * ENW + (386 - 64 * i) + q0
            src = _AP(e_dram.tensor, base,
                      [[4 * ENW - 1, 5], [ENW, 4], [1, q1 - q0]])
            eng = (nc.sync, nc.scalar)[i % 2]
            eng.dma_start(out=esh[20 * i:20 * i + 20, q0:q1], in_=src)
        for mm in range(m0, m1):
            wtile(mm)

    def tailnt(nt):
        zblk(nt)
        blk = slice(nt * 512, (nt + 1) * 512)
        nc.vector.tensor_tensor(enp[:, EPAD + nt * 512:EPAD + (nt + 1) * 512],
                                eraw[:, blk], zrep[:, blk], AL.mult)
        for sub in range(2):
            xtr(16 + nt * 2 + sub, on_act=True)
        if nt in (1, 4):
            shift_batch(nt)

    for nt in range(8):
        conv2(nt)
        tailnt(nt)
    shift_batch(7)

    st0 = st1 = None
    for ti in range(NTB):
        a = apool.tile([128, 3 * 512], fp16, tag="a")
        nc.gpsimd.local_scatter(
            out_ap=a[:], data_ap=wn[:, ti:ti + 3, :], idxs_ap=idxt[:],
            channels=128, num_elems=1536, num_idxs=300)
        if ti % NBLK == 0:
            st0 = stgp.tile([128, NBLK * 512], bf16, tag="st0")
            st1 = stgp.tile([64, NBLK * 512], bf16, tag="st1")
        q = ti % NBLK
        for ch in range(2):
            c0, cwid = (0, 128) if ch == 0 else (128, 64)
            ops = outps.tile([cwid, 512], fp32, tag=f"o{ch}")
            for kp in range(3):
                nc.tensor.matmul(ops[:], xT[:, ti + kp, c0:c0 + cwid],
                                 a[:, kp * 512:(kp + 1) * 512],
                                 start=(kp == 0), stop=(kp == 2))
            if ch == 0:
                nc.scalar.copy(out=st0[:, q * 512:(q + 1) * 512], in_=ops[:])
            else:
                nc.vector.tensor_scalar(st1[:, q * 512:(q + 1) * 512], ops[:],
                                        1.0, None, AL.mult)
        if q == NBLK - 1:
            u = ti // NBLK
            nc.sync.dma_start(
                out=out_d[0:128, u * 4 * NBLK:(u + 1) * 4 * NBLK, :],
                in_=st0[:].rearrange("c (b x) -> c b x", b=4 * NBLK))
            nc.sync.dma_start(
                out=out_d[128:192, u * 4 * NBLK:(u + 1) * 4 * NBLK, :],
                in_=st1[:].rearrange("c (b x) -> c b x", b=4 * NBLK))
    es.pop_all().close()


def _host_prep(inputs):
    def fold(w, g, b, m, v):
        s = g / np.sqrt(v + EPS)
        return (w * s[:, None, None, None]).astype(np.float32), (b - m * s).astype(np.float32)

    comp_w_eff, comp_b_eff = fold(inputs["comp_w"], inputs["comp_g"], inputs["comp_b"],
                                  inputs["comp_m"], inputs["comp_v"])
    enc_w_eff, enc_b_eff = fold(inputs["enc_w"], inputs["enc_g"], inputs["enc_b"],
                                inputs["enc_m"], inputs["enc_v"])
    cw = np.ascontiguousarray(comp_w_eff[:, :, 0, 0].T)          # [192, 64]
    ewm = np.concatenate([enc_w_eff[:, :, dy, dx].T
                          for dy in range(3) for dx in range(3)], axis=1)  # [64, 900]
    return dict(
        cw0=cw[0:128].astype(np.float16),
        cw1=cw[128:192].astype(np.float16),
        cb=comp_b_eff.reshape(Cm, 1).astype(np.float32),
        ew=np.ascontiguousarray(ewm).astype(np.float16),
        eb=enc_b_eff.reshape(E, 1).astype(np.float32),
    )


def kernel(**inputs):
    from concourse.bass_utils import run_bass_kernel_spmd

    inputs = {k: np.asarray(v, dtype=np.float32) for k, v in inputs.items()}
    w = _host_prep(inputs)
    if "nc" not in _prog_cache:
        _prog_cache["nc"] = _build_program()
    nc = _prog_cache["nc"]
    xh = inputs["x"].astype(np.float16)
    in_maps = [dict(x0=np.ascontiguousarray(xh[b, 0:128].reshape(128, PIX)),
                    x1=np.ascontiguousarray(xh[b, 128:192].reshape(64, PIX)),
                    **w) for b in range(B)]
    res = run_bass_kernel_spmd(nc, in_maps, list(range(B)))
    out = np.stack([np.asarray(res.results[b]["out"]).astype(np.float32)
                    for b in range(B)])
    return out


# revision 16
# speedup vs baseline: 1.0145x; 1.0145x over previous
# CARAFE Trainium2 Bass kernel, band-matrix formulation.
# Data-parallel over batch (8 items -> 8 NeuronCores). Per core:
#   - 1x1 compressor conv + folded BN + SiLU (PE fp16 matmuls, ACT sigmoid,
#     DVE fused mul)
#   - 3x3 encoder conv as 9 accumulating fp16 matmuls + folded BN, evicted
#     through ACT Exp -> unnormalized masks (bf16)
#   - mask normalization: per-pixel sum via a ones-selector matmul (PE),
#     reciprocal on DVE, replicated across the 100 mask channels by DMA
#   - per-tap-offset shifted mask copies (25 sbuf->sbuf DMAs into a padded
#     tile), transposed into source-pixel-major W tiles (PE)
#   - gpsimd local_scatter builds, per 128-pixel output block, the sparse
#     band matrix A[source_pixel, output_position] holding the 25 mask
#     diagonals; the whole 25-tap reassembly then collapses into 3
#     accumulating PE matmuls per (block, channel-chunk): out = x_pixmajor^T @ A
#   - PSUM evicted (ACT/DVE) to bf16 staging, DMA'd to DRAM channel-major.
import sys
import numpy as np

for _p in ("/opt/trn_rl_repo",):
    if _p not in sys.path:
        sys.path.insert(0, _p)

import ml_dtypes

B, C, Cm, E = 8, 192, 64, 100
H = W = 64
PIX = H * W              # 4096
K, S = 5, 2
EPS = 1e-3
NMM = 34                 # padded 128-pixel row tiles (mm = m + 1, m in [-1,33))
NTB = 32                 # output blocks of 2 lowres rows
EPAD = 384               # zero pad each side of the normalized-mask tile
ENW = EPAD + PIX + EPAD  # 4864
ESW = NMM * 128          # 4352
NBLK = 1                 # blocks staged per output DMA

_prog_cache = {}


def _off(i, j):
    return 64 * (i - 2) + (j - 2)


def _make_idx():
    idx = np.full((128, 300), -1, np.int16)
    for sp in range(128):
        for kp in range(3):
            for i in range(5):
                for j in range(5):
                    off = _off(i, j)
                    p_rel = 128 * (kp - 1) + sp - off
                    if not (0 <= p_rel < 128):
                        continue
                    r, w = divmod(p_rel, 64)
                    for cl in range(4):
                        di, dj = divmod(cl, 2)
                        col = kp * 100 + (i * 5 + j) * 4 + cl
                        idx[sp, col] = kp * 512 + ((r * 2 + di) * 64 + w) * 2 + dj
    return idx


def _build_program(num_devices=8):
    import concourse.mybir as mybir
    import concourse.tile as tile
    from concourse import bacc
    from contextlib import ExitStack

    fp32 = mybir.dt.float32
    fp16 = mybir.dt.float16
    bf16 = mybir.dt.bfloat16
    i16 = mybir.dt.int16

    nc = bacc.Bacc("TRN2", target_bir_lowering=False, num_devices=num_devices)

    x0_d = nc.dram_tensor("x0", [128, PIX], fp16, kind="ExternalInput").ap()
    x1_d = nc.dram_tensor("x1", [64, PIX], fp16, kind="ExternalInput").ap()
    cw0_d = nc.dram_tensor("cw0", [128, Cm], fp16, kind="ExternalInput").ap()
    cw1_d = nc.dram_tensor("cw1", [64, Cm], fp16, kind="ExternalInput").ap()
    cb_d = nc.dram_tensor("cb", [Cm, 1], fp32, kind="ExternalInput").ap()
    ew_d = nc.dram_tensor("ew", [Cm, 9 * E], fp16, kind="ExternalInput").ap()
    eb_d = nc.dram_tensor("eb", [E, 1], fp32, kind="ExternalInput").ap()
    out_d = nc.dram_tensor("out", [C, 2 * H, 2 * W], bf16, kind="ExternalOutput").ap()
    e_dram = nc.dram_tensor("edrm", [E, ENW], fp16, kind="Internal").ap()

    identh_t = nc.inline_tensor(np.eye(128, dtype=np.float16), name="identh").ap()
    sel_np = np.zeros((E, E), np.float32)
    for ci in range(E):
        for co in range(E):
            if ci % 4 == co % 4:
                sel_np[ci, co] = 1.0
    sel_t = nc.inline_tensor(sel_np.astype(ml_dtypes.bfloat16), name="selc").ap()
    edge_np = np.zeros((128, E), np.float32)
    for sp in range(128):
        for ch in range(E):
            j = (ch // 4) % 5
            wp_ = (sp - j + 2) % 64
            edge_np[sp, ch] = 1.0 if 0 <= wp_ + j - 2 < W else 0.0
    edge_t = nc.inline_tensor(edge_np.astype(np.float16), name="edgec").ap()
    idx_t = nc.inline_tensor(_make_idx(), name="idxc").ap()

    es = ExitStack()
    with tile.TileContext(nc) as tc:
        with es:
            _body(es, tc, nc, mybir, fp32, fp16, bf16, i16,
                  x0_d, x1_d, cw0_d, cw1_d, cb_d, ew_d, eb_d, out_d, e_dram,
                  identh_t, sel_t, edge_t, idx_t)
    nc.compile()
    return nc


def _body(es, tc, nc, mybir, fp32, fp16, bf16, i16,
          x0_d, x1_d, cw0_d, cw1_d, cb_d, ew_d, eb_d, out_d, e_dram,
          identh_t, sel_t, edge_t, idx_t):
    from contextlib import ExitStack
    AL = mybir.AluOpType
    AF = mybir.ActivationFunctionType

    consts = es.enter_context(tc.tile_pool(name="consts", bufs=1))
    big = es.enter_context(tc.tile_pool(name="big", bufs=1))

    identh = consts.tile([128, 128], fp16, tag="identh")
    sel = consts.tile([E, E], bf16, tag="sel")
    edge = consts.tile([128, E], fp16, tag="edge")
    idxt = consts.tile([128, 300], i16, tag="idxt")
    cw0 = consts.tile([128, Cm], fp16, tag="cw0")
    cw1 = consts.tile([64, Cm], fp16, tag="cw1")
    cb = consts.tile([Cm, 1], fp32, tag="cb")
    ew = consts.tile([Cm, 9 * E], fp16, tag="ew")
    eb = consts.tile([E, 1], fp32, tag="eb")

    x0 = big.tile([128, PIX], fp16, tag="x0")
    x1 = big.tile([64, PIX], fp16, tag="x1")
    xT = big.tile([128, NMM, C], fp16, tag="xT")
    tpad = big.tile([Cm, 66 * 66], fp16, tag="tpad")
    eraw = big.tile([E, PIX], bf16, tag="eraw")
    zrep = big.tile([E, PIX], bf16, tag="zrep")
    enp = big.tile([E, ENW], fp16, tag="enp")
    esh = big.tile([E, ESW], fp16, tag="esh")
    wn = big.tile([128, NMM, E], fp16, tag="wn")

    nc.scalar.dma_start(out=identh[:], in_=identh_t)
    nc.sync.dma_start(out=cw0[:], in_=cw0_d)
    nc.sync.dma_start(out=cw1[:], in_=cw1_d)
    nc.sync.dma_start(out=cb[:], in_=cb_d)
    nc.sync.dma_start(out=x0[:, 0:2048], in_=x0_d[:, 0:2048])
    nc.scalar.dma_start(out=x1[:, 0:2048], in_=x1_d[:, 0:2048])
    nc.sync.dma_start(out=x0[:, 2048:PIX], in_=x0_d[:, 2048:PIX])
    nc.scalar.dma_start(out=x1[:, 2048:PIX], in_=x1_d[:, 2048:PIX])
    nc.scalar.dma_start(out=ew[:], in_=ew_d)
    nc.scalar.dma_start(out=eb[:], in_=eb_d)
    nc.scalar.dma_start(out=sel[:], in_=sel_t)
    nc.scalar.dma_start(out=edge[:], in_=edge_t)
    nc.scalar.dma_start(out=idxt[:], in_=idx_t)

    nc.vector.memset(xT[:, 0, :], 0.0)
    nc.vector.memset(xT[:, NMM - 1, :], 0.0)
    tp3 = tpad[:].rearrange("c (r z) -> c r z", z=66)
    nc.vector.memset(tp3[:, 0:1, :], 0.0)
    nc.vector.memset(tp3[:, 65:66, :], 0.0)
    nc.vector.memset(tp3[:, 1:65, 0:1], 0.0)
    nc.vector.memset(tp3[:, 1:65, 65:66], 0.0)
    nc.vector.memset(enp[:, 0:EPAD], 0.0)
    nc.vector.memset(enp[:, EPAD + PIX:ENW], 0.0)

    c1ps = es.enter_context(tc.tile_pool(name="c1ps", bufs=2, space="PSUM"))
    c2ps = es.enter_context(tc.tile_pool(name="c2ps", bufs=2, space="PSUM"))
    trps = es.enter_context(tc.tile_pool(name="trps", bufs=2, space="PSUM"))
    outps = es.enter_context(tc.tile_pool(name="outps", bufs=1, space="PSUM"))
    apool = es.enter_context(tc.tile_pool(name="apool", bufs=6))
    stgp = es.enter_context(tc.tile_pool(name="stgp", bufs=4))

    def conv1(nt):
        n0 = nt * 512
        psf = c1ps.tile([128, 512], fp32, tag="c1")
        ps = psf[0:Cm, :]
        nc.tensor.matmul(ps, cw0[:], x0[:, n0:n0 + 512], start=True, stop=False)
        nc.tensor.matmul(ps, cw1[:], x1[:, n0:n0 + 512], start=False, stop=True)
        v = tp3[:, nt * 8 + 1:nt * 8 + 9, 1:65]
        nc.scalar.activation(out=v, in_=ps.rearrange("c (r z) -> c r z", z=64),
                             func=AF.Silu, bias=cb[:], scale=1.0)

    def conv2(nt):
        r0 = nt * 8
        ps = c2ps.tile([E, 512], fp32, tag="c2")
        for tap in range(9):
            dy, dx = divmod(tap, 3)
            rhs = tp3[:, r0 + dy:r0 + dy + 8, dx:dx + 64]
            nc.tensor.matmul(ps[:], ew[:, tap * E:(tap + 1) * E], rhs,
                             start=(tap == 0), stop=(tap == 8))
        nc.scalar.activation(out=eraw[:, nt * 512:(nt + 1) * 512], in_=ps[:],
                             func=AF.Exp, bias=eb[:], scale=1.0)

    def zblk(nt):
        pszf = c2ps.tile([E, 512], fp32, tag="c2")
        psz = pszf[:]
        nc.tensor.matmul(psz, sel[:], eraw[:, nt * 512:(nt + 1) * 512],
                         start=True, stop=True)
        with nc.allow_low_precision(reason="bf16 mask normalizer, 2e-2 tol"):
            nc.vector.reciprocal(zrep[:, nt * 512:(nt + 1) * 512], psz)

    def xtr(m, on_act=False):
        w0 = m * 128
        t0 = trps.tile([128, 128], fp16, tag="tp")
        nc.tensor.transpose(t0[:], x0[:, w0:w0 + 128], identh[:])
        t1f = trps.tile([128, 128], fp16, tag="tp")
        t1 = t1f[:, 0:64]
        nc.tensor.transpose(t1, x1[:, w0:w0 + 128], identh[0:64, 0:64])
        if on_act:
            nc.scalar.copy(out=xT[:, m + 1, 0:128], in_=t0[:])
            nc.scalar.copy(out=xT[:, m + 1, 128:192], in_=t1)
        else:
            nc.vector.tensor_scalar(xT[:, m + 1, 0:128], t0[:], 1.0, None, AL.mult)
            nc.vector.tensor_scalar(xT[:, m + 1, 128:192], t1, 1.0, None, AL.mult)

    for nt in range(8):
        conv1(nt)
        for sub in range(2):
            xtr(nt * 2 + sub)
    from concourse.ap import AP as _AP

    def wtile(mm):
        wpf = trps.tile([128, 128], fp16, tag="tp")
        wp = wpf[:, 0:E]
        nc.tensor.transpose(wp, esh[:, mm * 128:(mm + 1) * 128],
                            identh[0:E, 0:E])
        nc.vector.tensor_tensor(wn[:, mm, :], wp, edge[:], AL.mult)

    # staged shift roundtrip: (e_dram col range, esh q range, W mm range)
    BATCHES = {1: (0, 1408, 0, 896, 0, 7),
               4: (1408, 2944, 896, 2176, 7, 17),
               7: (2944, ENW, 2176, ESW, 17, NMM)}

    def shift_batch(nt):
        d0, d1, q0, q1, m0, m1 = BATCHES[nt]
        nc.sync.dma_start(out=e_dram[:, d0:d1], in_=enp[:, d0:d1])
        for i in range(5):
            base = (20 * i) * ENW + (386 - 64 * i) + q0
            src = _AP(e_dram.tensor, base,
                      [[4 * ENW - 1, 5], [ENW, 4], [1, q1 - q0]])
            eng = (nc.sync, nc.scalar)[i % 2]
            eng.dma_start(out=esh[20 * i:20 * i + 20, q0:q1], in_=src)
        for mm in range(m0, m1):
            wtile(mm)

    def tailnt(nt):
        zblk(nt)
        blk = slice(nt * 512, (nt + 1) * 512)
        nc.vector.tensor_tensor(enp[:, EPAD + nt * 512:EPAD + (nt + 1) * 512],
                                eraw[:, blk], zrep[:, blk], AL.mult)
        for sub in range(2):
            xtr(16 + nt * 2 + sub, on_act=True)
        if nt in (1, 4):
            shift_batch(nt)

    for nt in range(8):
        conv2(nt)
        tailnt(nt)
    shift_batch(7)

    st0 = st1 = None
    for ti in range(NTB):
        a = apool.tile([128, 3 * 512], fp16, tag="a")
        nc.gpsimd.local_scatter(
            out_ap=a[:], data_ap=wn[:, ti:ti + 3, :], idxs_ap=idxt[:],
            channels=128, num_elems=1536, num_idxs=300)
        if ti % NBLK == 0:
            st0 = stgp.tile([128, NBLK * 512], bf16, tag="st0")
            st1 = stgp.tile([64, NBLK * 512], bf16, tag="st1")
        q = ti % NBLK
        for ch in range(2):
            c0, cwid = (0, 128) if ch == 0 else (128, 64)
            ops = outps.tile([cwid, 512], fp32, tag=f"o{ch}")
            for kp in range(3):
                nc.tensor.matmul(ops[:], xT[:, ti + kp, c0:c0 + cwid],
                                 a[:, kp * 512:(kp + 1) * 512],
                                 start=(kp == 0), stop=(kp == 2))
            if ch == 0:
                nc.scalar.copy(out=st0[:, q * 512:(q + 1) * 512], in_=ops[:])
            else:
                nc.vector.tensor_scalar(st1[:, q * 512:(q + 1) * 512], ops[:],
                                        1.0, None, AL.mult)
        if q == NBLK - 1:
            u = ti // NBLK
            nc.sync.dma_start(
                out=out_d[0:128, u * 4 * NBLK:(u + 1) * 4 * NBLK, :],
                in_=st0[:].rearrange("c (b x) -> c b x", b=4 * NBLK))
            nc.sync.dma_start(
                out=out_d[128:192, u * 4 * NBLK:(u + 1) * 4 * NBLK, :],
                in_=st1[:].rearrange("c (b x) -> c b x", b=4 * NBLK))
    es.pop_all().close()


def _host_prep(inputs):
    def fold(w, g, b, m, v):
        s = g / np.sqrt(v + EPS)
        return (w * s[:, None, None, None]).astype(np.float32), (b - m * s).astype(np.float32)

    comp_w_eff, comp_b_eff = fold(inputs["comp_w"], inputs["comp_g"], inputs["comp_b"],
                                  inputs["comp_m"], inputs["comp_v"])
    enc_w_eff, enc_b_eff = fold(inputs["enc_w"], inputs["enc_g"], inputs["enc_b"],
                                inputs["enc_m"], inputs["enc_v"])
    cw = np.ascontiguousarray(comp_w_eff[:, :, 0, 0].T)          # [192, 64]
    ewm = np.concatenate([enc_w_eff[:, :, dy, dx].T
                          for dy in range(3) for dx in range(3)], axis=1)  # [64, 900]
    return dict(
        cw0=cw[0:128].astype(np.float16),
        cw1=cw[128:192].astype(np.float16),
        cb=comp_b_eff.reshape(Cm, 1).astype(np.float32),
        ew=np.ascontiguousarray(ewm).astype(np.float16),
        eb=enc_b_eff.reshape(E, 1).astype(np.float32),
    )


def kernel(**inputs):
    from concourse.bass_utils import run_bass_kernel_spmd

    inputs = {k: np.asarray(v, dtype=np.float32) for k, v in inputs.items()}
    w = _host_prep(inputs)
    if "nc" not in _prog_cache:
        _prog_cache["nc"] = _build_program()
    nc = _prog_cache["nc"]
    xh = inputs["x"].astype(np.float16)
    in_maps = [dict(x0=np.ascontiguousarray(xh[b, 0:128].reshape(128, PIX)),
                    x1=np.ascontiguousarray(xh[b, 128:192].reshape(64, PIX)),
                    **w) for b in range(B)]
    res = run_bass_kernel_spmd(nc, in_maps, list(range(B)))
    out = np.stack([np.asarray(res.results[b]["out"]).astype(np.float32)
                    for b in range(B)])
    return out
